# revision 1
# baseline (speedup 1.0000x reference)
"""Single-head causal attention (B=8, T=2048, C=768, H=64) on 8 TRN2 cores.

Sharding: data-parallel over batch — core i computes batch element i.
Inputs are cast to bf16 on the host (halves HBM traffic; matmul operands
must be bf16 for the 1 cycle/row PE rate anyway — fp32 runs at 1/4 rate).
Per-core pipeline (all on-chip after the x load):
  1. HWDGE DMA x [T, C] bf16 -> SBUF tiles [128, C]
  2. PE-transpose x -> xT [c=128 x 6, T] (projections contract over c)
  3. Projections (bf16, fp32 accum): packed [Wq|Wv] pass -> qT rows 0:64,
     vT rows 64:128 of one PSUM tile; separate Wk pass -> kT
  4. QK^T in transposed layout: weiT[tk, tq] = kT_blk.T @ qT_chunk, causal
     lower-triangle blocks only
  5. exp fused with PSUM eviction on ScalarE: expw = exp(0.125*(wei+mask)),
     bf16 out
  6. PV with ones-augmented v' [tk, 65]: outT'[0:64] = out^T, row 64 = row sums
  7. PE-transpose outT' -> [tq, 65], normalize cols 0:64 by col 64, DMA out
"""

import numpy as np

T, C, H = 2048, 768, 64
P = 128
NT = T // P        # 16 t-blocks
NCC = C // P       # 6 c-chunks
NJ = T // 512      # 4 tq chunks of 512
HP = H + 1         # 65: v plus ones column

_CACHE = {}


def _build():
    from contextlib import ExitStack

    import concourse.bacc as bacc
    import concourse.mybir as mybir
    import concourse.tile as tile
    from concourse.masks import make_identity

    f32 = mybir.dt.float32
    bf16 = mybir.dt.bfloat16
    AF = mybir.ActivationFunctionType

    nc = bacc.Bacc(None, target_bir_lowering=False, debug=False)

    x_d = nc.dram_tensor("x", [T, C], bf16, kind="ExternalInput")
    wq_d = nc.dram_tensor("Wq", [C, H], bf16, kind="ExternalInput")
    wk_d = nc.dram_tensor("Wk", [C, H], bf16, kind="ExternalInput")
    wv_d = nc.dram_tensor("Wv", [C, H], bf16, kind="ExternalInput")
    out_d = nc.dram_tensor("out", [T, H], f32, kind="ExternalOutput")

    with tile.TileContext(nc) as tc, ExitStack() as ctx:
        const = ctx.enter_context(tc.tile_pool(name="const", bufs=1))
        big = ctx.enter_context(tc.tile_pool(name="big", bufs=1))
        xp = ctx.enter_context(tc.tile_pool(name="xp", bufs=8))
        psA = ctx.enter_context(tc.tile_pool(name="psA", bufs=4, space="PSUM"))
        psW = ctx.enter_context(tc.tile_pool(name="psW", bufs=2, space="PSUM"))

        # --- constants ---
        ident = const.tile([P, P], bf16)
        make_identity(nc, ident[:])
        # identity on partitions 64..127 for transposing vT (which lives there)
        id64 = const.tile([P, H], bf16)
        make_identity(nc, id64[64:128, :])
        # f32 identity for the final [65, 128] transposes (outT is f32)
        id65 = const.tile([HP, HP], f32)
        make_identity(nc, id65[:])
        # triangular mask [128, 128]: 0 if f >= p else -1e10
        tri = const.tile([P, P], f32)
        nc.gpsimd.memset(tri[:], 0.0)
        nc.gpsimd.affine_select(
            out=tri[:], in_=tri[:],
            compare_op=mybir.AluOpType.is_ge,
            fill=-1e10,
            base=0,
            pattern=[[1, P]],
            channel_multiplier=-1,
        )

        # --- weights: packed [Wq | Wv] per c-chunk, plus Wk alone (bf16 cast) ---
        wqv = const.tile([P, NCC * P], bf16)   # chunk ci: cols [128ci,+64)=Wq, +64..128=Wv
        wk = const.tile([P, NCC * H], bf16)    # chunk ci: cols [64ci, 64ci+64)
        wqv_v = wqv[:].rearrange("p (ci r) -> p ci r", ci=NCC)
        nc.sync.dma_start(
            out=wqv_v[:, :, 0:H], in_=wq_d[:].rearrange("(ci p) h -> p ci h", p=P))
        nc.sync.dma_start(
            out=wqv_v[:, :, H : 2 * H], in_=wv_d[:].rearrange("(ci p) h -> p ci h", p=P))
        nc.sync.dma_start(
            out=wk[:].rearrange("p (ci h) -> p ci h", ci=NCC),
            in_=wk_d[:].rearrange("(ci p) h -> p ci h", p=P))

        # --- persistent SBUF tensors ---
        xT = big.tile([P, NCC * T], bf16)      # xT[:, T*ci + t]
        qvT = big.tile([P, T], bf16)           # rows 0:64 = qT, rows 64:128 = vT
        kT = big.tile([H, T], bf16)
        vp = big.tile([P, NT * HP], bf16)      # v' blocks: [tk, 64] + ones col
        expw = big.tile([P, 512 * 40], bf16)   # sum_j (4j+4) = 40 tiles of 512
        outT = big.tile([HP, T], f32)          # [65, 2048] pre-transpose output
        outsb = big.tile([P, NT * H], f32)     # final [t, h] tiles

        # expw column base offset for tq chunk j (4j+4 tiles of 512 each)
        def ew_base(j):
            return 512 * (2 * j * j + 2 * j)

        # --- fused per-chunk pipeline: load/transpose/project then attention ---
        for tj in range(NJ):
            for tb in range(4 * tj, 4 * tj + 4):
                xt = xp.tile([P, C], bf16, tag="xt")
                nc.sync.dma_start(out=xt[:], in_=x_d[P * tb : P * (tb + 1), :])
                # all 6 c-chunk transposes into one 1-bank PSUM tile
                pt = psA.tile([P, NCC * P], bf16, tag="ps")
                for ci in range(NCC):
                    nc.tensor.transpose(
                        pt[:, P * ci : P * (ci + 1)],
                        xt[:, P * ci : P * (ci + 1)],
                        ident[:],
                    )
                # one strided eviction per t-block; alternate DVE/ACT so
                # neither engine is the phase-A wall
                dst = xT[:].rearrange("p (ci t) -> p ci t", ci=NCC)[
                    :, :, P * tb : P * (tb + 1)
                ]
                src = pt[:].rearrange("p (q t) -> p q t", q=NCC)
                if tb % 2 == 0:
                    nc.vector.tensor_copy(dst, src)
                else:
                    nc.scalar.copy(dst, src)

            # qv projection for this 512-wide chunk
            pqv = psA.tile([P, 512], f32, tag="ps")
            for ci in range(NCC):
                nc.tensor.matmul(
                    pqv[:],
                    wqv[:, P * ci : P * (ci + 1)],
                    xT[:, T * ci + 512 * tj : T * ci + 512 * (tj + 1)],
                    start=(ci == 0),
                    stop=(ci == NCC - 1),
                )
            nc.vector.tensor_copy(qvT[:, 512 * tj : 512 * (tj + 1)], pqv[:])

            # k projection
            pk = psA.tile([H, 512], f32, tag="ps")
            for ci in range(NCC):
                nc.tensor.matmul(
                    pk[:],
                    wk[:, H * ci : H * (ci + 1)],
                    xT[:, T * ci + 512 * tj : T * ci + 512 * (tj + 1)],
                    start=(ci == 0),
                    stop=(ci == NCC - 1),
                )
            nc.scalar.copy(kT[:, 512 * tj : 512 * (tj + 1)], pk[:])

            # v' blocks for this chunk: transpose vT (rows 64:128 of qvT) to [tk, 64]
            for tb in range(4 * tj, 4 * tj + 4):
                pv = psA.tile([P, H], bf16, tag="ps")
                nc.tensor.transpose(
                    pv[:],
                    qvT[64:128, P * tb : P * (tb + 1)],
                    id64[64:128, :],
                )
                nc.vector.tensor_copy(vp[:, HP * tb : HP * tb + H], pv[:])
                nc.gpsimd.memset(vp[:, HP * tb + H : HP * (tb + 1)], 1.0)

        # --- phase B: attention per tq chunk ---
        for j in range(NJ):
            ntk = 4 * j + 4
            for half in range(ntk // 2):
                pw = psW.tile([P, 1024], f32, tag="pw")
                for s in range(2):
                    tkb = 2 * half + s
                    nc.tensor.matmul(
                        pw[:, 512 * s : 512 * (s + 1)],
                        kT[:, P * tkb : P * (tkb + 1)],
                        qvT[0:64, 512 * j : 512 * (j + 1)],
                        start=True,
                        stop=True,
                    )
                    d = tkb - 4 * j
                    if d >= 0:  # diagonal block: causal tri-mask on its 128 cols
                        blk = pw[:, 512 * s + P * d : 512 * s + P * (d + 1)]
                        nc.vector.tensor_add(blk, blk, tri[:])
                # fused scale + exp, PSUM -> SBUF bf16
                base = ew_base(j) + 1024 * half
                nc.scalar.activation(
                    expw[:, base : base + 1024], pw[:], AF.Exp, scale=0.125)

            # PV: accumulate over tk blocks; out rows 0:64 = out^T, row 64 = sums
            po = psA.tile([HP, 512], f32, tag="ps")
            for tkb in range(ntk):
                d = tkb - 4 * j
                skip = P * d if d > 0 else 0
                nc.tensor.matmul(
                    po[:, skip:512],
                    vp[:, HP * tkb : HP * tkb + HP],
                    expw[:, ew_base(j) + 512 * tkb + skip : ew_base(j) + 512 * (tkb + 1)],
                    start=(tkb == 0),
                    stop=(tkb == ntk - 1),
                )
            nc.vector.tensor_copy(outT[:, 512 * j : 512 * (j + 1)], po[:])

            # transpose back to [tq, 65] and normalize
            for i in range(4):
                tb = 4 * j + i
                pt = psA.tile([P, HP], f32, tag="ps")
                nc.tensor.transpose(
                    pt[:],
                    outT[:, P * tb : P * (tb + 1)],
                    id65[:],
                )
                rc = xp.tile([P, 1], f32, tag="rc")
                nc.vector.reciprocal(rc[:], pt[:, H : H + 1])
                nc.vector.tensor_scalar_mul(
                    outsb[:, H * tb : H * (tb + 1)], pt[:, 0:H], rc[:])

            # stream this chunk's output to DRAM while later chunks compute
            nc.sync.dma_start(
                out=out_d[512 * j : 512 * (j + 1)].rearrange(
                    "(tb p) h -> p tb h", p=P),
                in_=outsb[:].rearrange("p (tb h) -> p tb h", tb=NT)[
                    :, 4 * j : 4 * (j + 1), :],
            )


    nc.compile()
    return nc


def _get_nc():
    if "nc" not in _CACHE:
        _CACHE["nc"] = _build()
    return _CACHE["nc"]


def kernel(x, Wk, Wq, Wv):
    import ml_dtypes

    from concourse.bass_utils import run_bass_kernel_spmd

    bf = ml_dtypes.bfloat16
    x = np.ascontiguousarray(np.asarray(x, dtype=np.float32).astype(bf))
    Wk = np.ascontiguousarray(np.asarray(Wk, dtype=np.float32).astype(bf))
    Wq = np.ascontiguousarray(np.asarray(Wq, dtype=np.float32).astype(bf))
    Wv = np.ascontiguousarray(np.asarray(Wv, dtype=np.float32).astype(bf))
    B = x.shape[0]
    nc = _get_nc()
    in_maps = [
        {"x": np.ascontiguousarray(x[b]), "Wq": Wq, "Wk": Wk, "Wv": Wv}
        for b in range(B)
    ]
    res = run_bass_kernel_spmd(nc, in_maps, core_ids=list(range(B)))
    return np.stack([res.results[b]["out"] for b in range(B)])

